# revision 1
# baseline (speedup 1.0000x reference)
"""Trainium2 Bass kernel for nn_BasicDeconvolutionBlock.

Reference computation (see problem statement):
    gathered = feats[in_map]                         # [K, M, Cin]
    contrib  = einsum('kmc,kcd->kmd', gathered, W)   # [K, M, Cout]
    out      = zeros([n_out, Cout]).at[out_map].add(contrib)
    y        = relu(batchnorm(out))                  # batch stats over n_out rows

Strategy (8 NeuronCores, SPMD):
  - Host routes each (k, m) pair to the core owning its output row
    (row blocks of n_out/8).  Per core ~169k pairs.
  - Gather: feats pre-cast to bf16, padded to 128 channels (256B rows).
    dma_gather(transpose=True) produces a CHANNEL-MAJOR SBUF slab
    G[128ch, slots] directly.  int16 gather indices -> feats is split in
    chunks of 32768 rows; pairs are grouped by (chunk, k), groups padded
    to a multiple of 128 slots.
  - GEMM: per 128-slot tile, matmul(lhsT=G_tile[128ch,128slots] (stationary),
    rhs=Wpad[k][128ch,64]) -> PSUM contrib[128slots, 64] fp32 (m-major,
    no transposes anywhere).
  - Scatter: DVE copies PSUM->SBUF slab, then gpsimd dma_scatter_add
    (CCE-add, int16 idx) accumulates rows into one of two HBM accumulator
    banks (cycled by round parity so chains overlap).  Duplicate rows race
    in hardware, so a host-side occurrence-round split guarantees unique
    rows per call; same-bank calls serialize via Tile WAW deps.  SWDGE
    calls are capped at 896 indices (the Q7 ucode descriptor-ring limit;
    larger calls hard-wedge the device).
  - BN: ones-matmul row sums + sum of squares, [2,64] AllReduce across
    the 8 cores, normalize + ReLU on chip, output shard [rows,64] fp32.
"""

import os
import sys

import numpy as np

sys.path.insert(0, "/opt/trn_rl_repo")

import ml_dtypes  # noqa: E402

from concourse import bacc, bass, mybir  # noqa: E402
import concourse.tile as tile  # noqa: E402

BN_EPS = 1e-5
CHUNK = 32768  # int16 gather index range per feats chunk
SEG_SLOTS = 896  # max slots per SWDGE call; 1024+ wedges the device (Q7 ucode descriptor-ring limit, verified empirically)
F32 = mybir.dt.float32
BF16 = mybir.dt.bfloat16
I16 = mybir.dt.int16
I32 = mybir.dt.int32


def _roundup(x, m):
    return (x + m - 1) // m * m


def _route(in_map, out_map, n_out, n_cores, dup_safe, expand=1):
    """Host-side routing. Returns compile-time plan + per-core packed arrays.

    Slot stream per core: for r in rounds, for c in chunks, for k in K:
    group (r,c,k) padded to a multiple of 128 slots.  If dup_safe, a single
    round (r=0) is used (occurrence splitting disabled).

    expand=E spreads a row's duplicate contributions over E contiguous
    accumulator banks (phys row = (occ%E)*acc_rows + row, round = occ//E),
    halving/quartering the round count; the kernel folds banks before BN.
    """
    K, M = in_map.shape
    rows_per_core = n_out // n_cores
    assert rows_per_core * n_cores == n_out
    acc_rows = _roundup(rows_per_core, 128)
    nchunk = _roundup(int(in_map.max()) + 1, CHUNK) // CHUNK

    k_idx = np.repeat(np.arange(K, dtype=np.int32), M)
    in_flat = in_map.ravel().astype(np.int64)
    out_flat = out_map.ravel().astype(np.int64)
    core = out_flat // rows_per_core
    row_local = (out_flat - core * rows_per_core).astype(np.int32)
    chunk = (in_flat // CHUNK).astype(np.int32)
    idx_local = (in_flat - chunk.astype(np.int64) * CHUNK).astype(np.int32)

    per_core = []
    max_round = 1
    for c in range(n_cores):
        sel = np.nonzero(core == c)[0]
        rows_c = row_local[sel]
        if dup_safe:
            rnd = np.zeros(len(sel), dtype=np.int32)
            prow = rows_c.astype(np.int32)
        else:
            order = np.argsort(rows_c, kind="stable")
            sr = rows_c[order]
            n = len(sr)
            first = np.ones(n, dtype=bool)
            first[1:] = sr[1:] != sr[:-1]
            grp_start = np.maximum.accumulate(np.where(first, np.arange(n), 0))
            occ_sorted = np.arange(n) - grp_start
            occ = np.empty(n, dtype=np.int64)
            occ[order] = occ_sorted
            rnd = (occ // expand).astype(np.int32)
            prow = (rows_c + (occ % expand) * acc_rows).astype(np.int32)
            max_round = max(max_round, int(rnd.max()) + 1 if n else 1)
        per_core.append(
            dict(rnd=rnd, chunk=chunk[sel], k=k_idx[sel],
                 idx=idx_local[sel], row=prow)
        )

    R = max_round
    # group counts [R, nchunk, K] per core -> shared caps
    counts = np.zeros((n_cores, R, nchunk, K), dtype=np.int64)
    for c in range(n_cores):
        p = per_core[c]
        np.add.at(counts[c], (p["rnd"], p["chunk"], p["k"]), 1)
    caps = (np.ceil(counts.max(axis=0) / 128).astype(np.int64) * 128)  # [R,nchunk,K]

    # segments: contiguous runs of (r,c,k) group pieces, same (r,c),
    # <= SEG_SLOTS per segment (SWDGE per-instruction descriptor limit).
    # Groups larger than SEG_SLOTS are split across segments.
    segments = []  # dicts: r, c, slot0 (global), nslots, groups=[(k, len, off_in_seg)]
    group_slot0 = {}  # (r,c,k) -> global slot of the group's first slot
    slot0 = 0
    for r in range(R):
        for c in range(nchunk):
            cur = None
            for k in range(K):
                cap = int(caps[r, c, k])
                if cap == 0:
                    continue
                group_slot0[(r, c, k)] = slot0 + (cur["nslots"] if cur else 0)
                rem = cap
                while rem > 0:
                    if cur is None:
                        cur = dict(r=r, c=c, slot0=slot0, nslots=0, groups=[])
                    take = min(SEG_SLOTS - cur["nslots"], rem)
                    if take == 0:
                        segments.append(cur)
                        slot0 += cur["nslots"]
                        cur = None
                        continue
                    cur["groups"].append((k, take, cur["nslots"]))
                    cur["nslots"] += take
                    rem -= take
            if cur is not None:
                segments.append(cur)
                slot0 += cur["nslots"]
                cur = None
    total_slots = slot0

    dump_row = expand * acc_rows  # rows beyond the banks are the dump zone
    acc_total = expand * acc_rows + 128

    # pack per-core gather idx and scatter idx (both int16, wrapped 16)
    gcols = sum(seg["nslots"] // 16 for seg in segments)
    scols = gcols
    gidx_all = np.zeros((n_cores, 128, gcols), dtype=np.int16)
    sidx_all = np.full((n_cores, 128, scols), dump_row, dtype=np.int16)

    seg_gcol0 = []
    seg_scol0 = []
    g0 = s0 = 0
    for seg in segments:
        seg_gcol0.append(g0)
        seg_scol0.append(s0)
        g0 += seg["nslots"] // 16
        s0 += seg["nslots"] // 16

    for cidx in range(n_cores):
        p = per_core[cidx]
        order = np.lexsort((p["row"], p["k"], p["chunk"], p["rnd"]))
        rnd_s, ch_s, k_s = p["rnd"][order], p["chunk"][order], p["k"][order]
        idx_s, row_s = p["idx"][order], p["row"][order]
        # slot of each pair: group_slot0 + position within group
        key = (rnd_s.astype(np.int64) * nchunk + ch_s) * K + k_s
        n = len(key)
        first = np.ones(n, dtype=bool)
        first[1:] = key[1:] != key[:-1]
        grp_start = np.maximum.accumulate(np.where(first, np.arange(n), 0))
        pos_in_grp = np.arange(n) - grp_start
        base = np.array(
            [group_slot0[(int(r_), int(c_), int(k_))]
             for r_, c_, k_ in zip(rnd_s[first], ch_s[first], k_s[first])],
            dtype=np.int64,
        )
        base_full = np.repeat(base, np.diff(np.nonzero(
            np.concatenate([first, [True]]))[0]))
        slots = base_full + pos_in_grp

        gvals = np.zeros(total_slots, dtype=np.int16)
        svals = np.full(total_slots, dump_row, dtype=np.int16)
        gvals[slots] = idx_s.astype(np.int16)
        svals[slots] = row_s
        # per-segment packing
        for si, seg in enumerate(segments):
            a, b = seg["slot0"], seg["slot0"] + seg["nslots"]
            gseg = gvals[a:b].reshape(-1, 16).T  # [16, n/16]
            gidx_all[cidx, :, seg_gcol0[si]:seg_gcol0[si] + (b - a) // 16] = (
                np.tile(gseg, (8, 1)))
            sseg = np.tile(svals[a:b].astype(np.int16).reshape(-1, 16).T,
                           (8, 1))  # wrapped like gather idxs
            sidx_all[cidx, :, seg_scol0[si]:seg_scol0[si] + (b - a) // 16] = sseg

    plan = dict(
        R=R, nchunk=nchunk, K=K, rows_per_core=rows_per_core,
        acc_rows=acc_rows, acc_total=acc_total, dump_row=dump_row,
        expand=expand,
        segments=segments, seg_gcol0=seg_gcol0, seg_scol0=seg_scol0,
        gcols=gcols, scols=scols, total_slots=total_slots,
    )
    return plan, gidx_all, sidx_all


def _build(plan, n_out, ftab_rows, n_cores):
    """Trace the Bass program. Returns nc."""
    nc = bacc.Bacc("TRN2", target_bir_lowering=False, debug=False)

    R, nchunk, K = plan["R"], plan["nchunk"], plan["K"]
    acc_rows, acc_total = plan["acc_rows"], plan["acc_total"]
    segments = plan["segments"]
    Cout = 64

    ftab = nc.dram_tensor("ftab", [ftab_rows, 128], BF16, kind="ExternalInput")
    wt = nc.dram_tensor("wt", [128, K * Cout], BF16, kind="ExternalInput")
    gidx = nc.dram_tensor("gidx", [128, plan["gcols"]], I16, kind="ExternalInput")
    sidx = nc.dram_tensor("sidx", [128, plan["scols"]], I16, kind="ExternalInput")
    gb = nc.dram_tensor("gb", [2, Cout], F32, kind="ExternalInput")
    # two accumulator banks cycled by round parity: scatter calls to
    # different banks have no WAW conflict, so adjacent rounds overlap
    acc0 = nc.dram_tensor("acc0", [acc_total, Cout], F32)
    acc1 = nc.dram_tensor("acc1", [acc_total, Cout], F32)
    accs = [acc0, acc1]
    cc_in = nc.dram_tensor("cc_in", [2, Cout], F32)
    cc_out = nc.dram_tensor("cc_out", [2, Cout], F32, addr_space="Shared")
    y = nc.dram_tensor("y", [acc_rows, Cout], F32, kind="ExternalOutput")

    Tb = acc_rows // 128  # BN column tiles

    with tile.TileContext(nc) as tc:
        with (
            tc.tile_pool(name="const", bufs=1) as cpool,
            tc.tile_pool(name="gpool", bufs=3) as gpool,
            tc.tile_pool(name="slab", bufs=3) as slabpool,
            tc.tile_pool(name="gixp", bufs=3) as gixpool,
            tc.tile_pool(name="sixp", bufs=3) as sixpool,
            tc.tile_pool(name="psum", bufs=8, space="PSUM") as pspool,
        ):
            # constants
            w_sb = cpool.tile([128, K * Cout], BF16, tag="w")
            nc.sync.dma_start(out=w_sb[:, :], in_=wt[:, :])
            zed = cpool.tile([128, 3200], F32, tag="zed")
            nc.vector.memset(zed[:, :], 0.0)
            # zero-init acc (acc_total*64 elems, in chunks of 128*3200)
            zrows = 128 * 3200 // Cout  # 6400 rows per DMA
            for bank in accs:
                r0 = 0
                while r0 < acc_total:
                    rcnt = min(zrows, acc_total - r0)
                    nc.sync.dma_start(
                        out=bank[r0:r0 + rcnt, :],
                        in_=zed[:, :rcnt * Cout // 128],
                    )
                    r0 += rcnt

            # main pipeline over segments
            for si, seg in enumerate(segments):
                ns = seg["nslots"]
                c = seg["c"]
                gi = gixpool.tile([128, SEG_SLOTS // 16], I16, tag="gi")
                nc.sync.dma_start(
                    out=gi[:, :ns // 16],
                    in_=gidx[:, plan["seg_gcol0"][si]:plan["seg_gcol0"][si] + ns // 16],
                )
                g = gpool.tile([128, 1, SEG_SLOTS], BF16, tag="g")
                nc.gpsimd.dma_gather(
                    out_ap=g[:, :, :ns],
                    in_ap=ftab[c * CHUNK:min((c + 1) * CHUNK, ftab_rows), :],
                    idxs_ap=gi[:, :ns // 16],
                    num_idxs=ns,
                    num_idxs_reg=ns,
                    elem_size=128,
                    transpose=True,
                )
                slab = slabpool.tile([128, SEG_SLOTS // 128, Cout], F32, tag="slab")
                for (k, cap, off) in seg["groups"]:
                    for j in range(cap // 128):
                        col = off + j * 128
                        ps = pspool.tile([128, Cout], F32, tag="ps")
                        nc.tensor.matmul(
                            out=ps[:, :],
                            lhsT=g[:, 0, col:col + 128],
                            rhs=w_sb[:, k * Cout:(k + 1) * Cout],
                            start=True, stop=True,
                        )
                        nc.vector.tensor_copy(
                            out=slab[:, col // 128, :], in_=ps[:, :])
                si_t = sixpool.tile([128, SEG_SLOTS // 16], I16, tag="si")
                nc.sync.dma_start(
                    out=si_t[:, :ns // 16],
                    in_=sidx[:, plan["seg_scol0"][si]:plan["seg_scol0"][si] + ns // 16],
                )
                nc.gpsimd.dma_scatter_add(
                    out_ap=accs[seg["r"] % 2][:, :],
                    in_ap=slab[:, :ns // 128, :],
                    idxs_ap=si_t[:, :ns // 16],
                    num_idxs=ns,
                    num_idxs_reg=ns,
                    elem_size=64,
                )

        # ---- BN phase ----
        with (
            tc.tile_pool(name="bn", bufs=1) as bnpool,
            tc.tile_pool(name="bns", bufs=4) as bnspool,
            tc.tile_pool(name="bnp", bufs=2, space="PSUM") as bnps,
        ):
            out_sb = bnpool.tile([128, Tb, 64], F32, tag="outsb")
            nc.sync.dma_start(out=out_sb[:, :, :], in_=acc0[0:acc_rows, :])
            bank_sb = bnpool.tile([128, Tb, 64], F32, tag="bank")
            nc.sync.dma_start(out=bank_sb[:, :, :], in_=acc1[0:acc_rows, :])
            nc.vector.tensor_tensor(
                out=out_sb[:, :, :], in0=out_sb[:, :, :],
                in1=bank_sb[:, :, :], op=mybir.AluOpType.add)
            ones = bnpool.tile([128, 1], F32, tag="ones")
            nc.vector.memset(ones[:, :], 1.0)
            sum_ps = bnps.tile([1, 64], F32, tag="sum")
            sq_ps = bnps.tile([1, 64], F32, tag="sq")
            for t in range(Tb):
                nc.tensor.matmul(
                    out=sum_ps[:, :], lhsT=ones[:, :], rhs=out_sb[:, t, :],
                    start=(t == 0), stop=(t == Tb - 1),
                )
            sqt = bnspool.tile([128, 64], F32, tag="sqt")
            for t in range(Tb):
                nc.vector.tensor_tensor(
                    out=sqt[:, :], in0=out_sb[:, t, :], in1=out_sb[:, t, :],
                    op=mybir.AluOpType.mult)
                nc.tensor.matmul(
                    out=sq_ps[:, :], lhsT=ones[:, :], rhs=sqt[:, :],
                    start=(t == 0), stop=(t == Tb - 1),
                )
            st0 = bnspool.tile([1, 64], F32, tag="st0")
            st1 = bnspool.tile([1, 64], F32, tag="st1")
            nc.vector.tensor_copy(out=st0[:, :], in_=sum_ps[:, :])
            nc.vector.tensor_copy(out=st1[:, :], in_=sq_ps[:, :])
            nc.sync.dma_start(out=cc_in[0:1, :], in_=st0[:, :])
            nc.sync.dma_start(out=cc_in[1:2, :], in_=st1[:, :])
            nc.gpsimd.collective_compute(
                "AllReduce",
                mybir.AluOpType.add,
                ins=[cc_in[:, :]],
                outs=[cc_out[:, :]],
                replica_groups=[list(range(n_cores))],
            )
            gs0 = bnspool.tile([1, 64], F32, tag="gs0")
            gs1 = bnspool.tile([1, 64], F32, tag="gs1")
            nc.sync.dma_start(out=gs0[:, :], in_=cc_out[0:1, :])
            nc.sync.dma_start(out=gs1[:, :], in_=cc_out[1:2, :])
            gam_t = bnspool.tile([1, 64], F32, tag="gam")
            bet_t = bnspool.tile([1, 64], F32, tag="bet")
            nc.sync.dma_start(out=gam_t[:, :], in_=gb[0:1, :])
            nc.sync.dma_start(out=bet_t[:, :], in_=gb[1:2, :])

            inv_n = 1.0 / float(n_out)
            mean_t = bnspool.tile([1, 64], F32, tag="mean")
            ex2_t = bnspool.tile([1, 64], F32, tag="ex2")
            var_t = bnspool.tile([1, 64], F32, tag="var")
            sd_t = bnspool.tile([1, 64], F32, tag="sd")
            rs_t = bnspool.tile([1, 64], F32, tag="rs")
            a_t = bnspool.tile([1, 64], F32, tag="a")
            b_t = bnspool.tile([1, 64], F32, tag="b")
            nc.vector.tensor_scalar_mul(mean_t[:, :], gs0[:, :], inv_n)
            nc.vector.tensor_scalar_mul(ex2_t[:, :], gs1[:, :], inv_n)
            nc.vector.tensor_tensor(
                out=var_t[:, :], in0=mean_t[:, :], in1=mean_t[:, :],
                op=mybir.AluOpType.mult)
            nc.vector.tensor_tensor(
                out=var_t[:, :], in0=ex2_t[:, :], in1=var_t[:, :],
                op=mybir.AluOpType.subtract)
            nc.vector.tensor_scalar_add(var_t[:, :], var_t[:, :], BN_EPS)
            nc.scalar.activation(
                out=sd_t[:, :], in_=var_t[:, :],
                func=mybir.ActivationFunctionType.Sqrt)
            nc.vector.reciprocal(out=rs_t[:, :], in_=sd_t[:, :])
            nc.vector.tensor_tensor(
                out=a_t[:, :], in0=gam_t[:, :], in1=rs_t[:, :],
                op=mybir.AluOpType.mult)
            nc.vector.tensor_tensor(
                out=b_t[:, :], in0=mean_t[:, :], in1=a_t[:, :],
                op=mybir.AluOpType.mult)
            nc.vector.tensor_tensor(
                out=b_t[:, :], in0=bet_t[:, :], in1=b_t[:, :],
                op=mybir.AluOpType.subtract)
            # broadcast [1,64] -> [128,64] via PE (ones[1,128]^T @ row)
            ones_row = bnspool.tile([1, 128], F32, tag="ones_row")
            nc.vector.memset(ones_row[:, :], 1.0)
            a_full = bnspool.tile([128, 64], F32, tag="afull")
            b_full = bnspool.tile([128, 64], F32, tag="bfull")
            ab_ps = bnps.tile([128, 64], F32, tag="abps")
            nc.tensor.matmul(
                out=ab_ps[:, :], lhsT=ones_row[:, :], rhs=a_t[:, :],
                start=True, stop=True)
            nc.vector.tensor_copy(out=a_full[:, :], in_=ab_ps[:, :])
            nc.tensor.matmul(
                out=ab_ps[:, :], lhsT=ones_row[:, :], rhs=b_t[:, :],
                start=True, stop=True)
            nc.vector.tensor_copy(out=b_full[:, :], in_=ab_ps[:, :])
            for t in range(Tb):
                nc.vector.tensor_tensor(
                    out=out_sb[:, t, :], in0=out_sb[:, t, :], in1=a_full[:, :],
                    op=mybir.AluOpType.mult)
                nc.vector.tensor_tensor(
                    out=out_sb[:, t, :], in0=out_sb[:, t, :], in1=b_full[:, :],
                    op=mybir.AluOpType.add)
                nc.scalar.activation(
                    out=out_sb[:, t, :], in_=out_sb[:, t, :],
                    func=mybir.ActivationFunctionType.Relu)
            nc.sync.dma_start(out=y[:, :], in_=out_sb[:, :, :])

    nc.compile()
    return nc


def _prepare(feats, W, gamma, beta, in_map, out_map, n_out, n_cores, dup_safe,
             expand=1):
    """Host prep shared by kernel() and tests. Returns (nc, in_maps, plan)."""
    n_out = int(n_out)
    K, Cin, Cout = W.shape
    assert Cin == 64 and Cout == 64
    in_map = np.asarray(in_map, dtype=np.int64)
    out_map = np.asarray(out_map, dtype=np.int64)
    feats = np.asarray(feats, dtype=np.float32)
    W = np.asarray(W, dtype=np.float32)

    plan, gidx_all, sidx_all = _route(
        in_map, out_map, n_out, n_cores, dup_safe, expand)

    ftab_rows = _roundup(feats.shape[0], CHUNK)
    ftab = np.zeros((ftab_rows, 128), dtype=ml_dtypes.bfloat16)
    ftab[:feats.shape[0], :64] = feats.astype(ml_dtypes.bfloat16)

    # W padded: [128 ic, K*64] bf16, rows 64..127 zero
    wt = np.zeros((128, K * 64), dtype=ml_dtypes.bfloat16)
    wt[:64, :] = (
        W.transpose(1, 0, 2).reshape(64, K * 64).astype(ml_dtypes.bfloat16))

    gb = np.stack([np.asarray(gamma, np.float32),
                   np.asarray(beta, np.float32)])

    nc = _build(plan, n_out, ftab_rows, n_cores)
    in_maps = [
        dict(ftab=ftab, wt=wt, gidx=gidx_all[c], sidx=sidx_all[c], gb=gb)
        for c in range(n_cores)
    ]
    return nc, in_maps, plan


def kernel(feats, W, gamma, beta, in_map, out_map, n_out):
    from concourse.bass_utils import run_bass_kernel_spmd

    n_cores = 8
    dup_safe = os.environ.get("DECONV_DUP_SAFE", "0") == "1"
    expand = int(os.environ.get("DECONV_EXPAND", "1"))
    nc, in_maps, plan = _prepare(
        feats, W, gamma, beta, in_map, out_map, n_out, n_cores, dup_safe,
        expand)
    res = run_bass_kernel_spmd(nc, in_maps, list(range(n_cores)))
    rows = plan["rows_per_core"]
    out = np.concatenate(
        [res.results[c]["y"][:rows] for c in range(n_cores)], axis=0)
    return out.astype(np.float32)

